# revision 31
# baseline (speedup 1.0000x reference)
"""Trainium2 Bass kernel for nn_Attention_64819646431478.

Single-layer causal attention, B=1, T=2048, DIM=1024, 16 heads, head_dim=64,
f32, with RMSNorm (eps=f32 eps) on Q and K heads.

Sharding: tensor-parallel over heads across 8 NeuronCores (2 heads/core).
Each core computes its heads' Q/K/V projections, causal attention, and the
partial output projection against its 128-row slice of w_o; the host sums
the 8 partial outputs (the "all-reduce" of the hint, done at gather time).

v3 design notes (vs the v2 baseline at 132us):
  - ONE ACT table set for the whole kernel (natural_log_exp_and_others):
    softmax exp, plus RMS rsqrt computed as exp(-0.5*ln(ms+eps)).  No
    mid-kernel ACT_TABLE_LOADs, so no PE stall at the B->C boundary.
  - Chunk-interleaved emission: projections for chunk c+1 and the output
    projection for chunk c-1 are emitted *between* the attention groups of
    chunk c, so the PE queue always has dependency-free matmuls to run
    while ACT streams the exp of the current groups.  This keeps the PE
    HAM-warm (2.4 GHz) end to end.
  - V transpose moved off the PE/DVE onto the DMA xbar transpose engine.
  - DVE diet: squares from the bf16 raw copy (tensor_mul), rinv/sg128 rows
    written in place (no rec/sgf shuffling), big memsets on gpsimd, every
    4th causal-mask multiply on gpsimd.
  - xT is chunk-major in dram; chunk 0 arrives as 8 per-ci pieces so the
    first projection matmul starts ~1us in.
  - PSUM: st 3x[128,512] + ot 2 + shared(pp/sums/bb/wo/b2) 3 = 8 banks,
    sized so the interleaved B/C/wo streams coexist.
"""

import os
import sys
import types
from collections import deque

import numpy as np

# --- environment bootstrap (harness may run us from a bare directory) ---
for _p in ("/root/.axon_site", "/root/.axon_site/_ro/trn_rl_repo",
           "/root/.axon_site/_ro/pypackages", "/opt/trn_rl_repo"):
    if os.path.isdir(_p) and _p not in sys.path:
        sys.path.append(_p)


def _install_ntff_shim():
    """Provide antenv.axon_hooks (missing in this image) so trace=True works."""
    if "antenv.axon_hooks" in sys.modules:
        return
    mod = types.ModuleType("antenv.axon_hooks")
    mod._hook = None
    mod.set_axon_ntff_profile_hook = lambda h: setattr(mod, "_hook", h)
    mod.get_axon_ntff_profile_hook = lambda: mod._hook
    sys.modules["antenv.axon_hooks"] = mod
    try:
        import antenv
        antenv.axon_hooks = mod
        from trn_agent_boot.trn_boot import _ntff_profile_via_ctypes
        mod.set_axon_ntff_profile_hook(
            _ntff_profile_via_ctypes("/opt/axon/libaxon_pjrt.so"))
    except Exception:
        pass


_install_ntff_shim()

import ml_dtypes  # noqa: E402

import concourse.mybir as mybir  # noqa: E402
import concourse.tile as tile  # noqa: E402
from concourse import bacc  # noqa: E402

F32 = mybir.dt.float32
BF16 = mybir.dt.bfloat16
NP_BF16 = ml_dtypes.bfloat16
AF = mybir.ActivationFunctionType
MUL = mybir.AluOpType.mult

T = 2048
C = 1024
D = 64
NCORES = 8
HPC = 2            # heads per core
JPC = HPC * D      # 128 j-columns per core
NTQ = 4            # tq chunks of 512
TQ = 512
NTK = 16           # tk tiles of 128
EPS = float(np.finfo(np.float32).eps)
SCALE = float(D) ** -0.5

USE_DMA_TRANSPOSE = False  # V transpose on DMA xbar instead of PE
MASK_GP_EVERY = 10 ** 9    # every Nth mask multiply goes to gpsimd
STG_ACT_EVERY = 4          # every Nth wo staging copy goes to ACT


def _pin_act_table_set(arch):
    """Steer the ACT table-set chooser to natural_log_exp_and_others.

    The kernel only uses Exp, Ln and Copy on the scalar engine, and one set
    contains all three.  The default chooser binds Exp to exp_and_others and
    Ln to natural_log, which thrashes ~2.7us of ACT_TABLE_LOAD per RMS
    instance.  Emptying the *contents* of the competing cached set entries
    (order and indices unchanged, so the runtime set ids stay correct) makes
    the fixpoint pass pick the combined set everywhere -> a single load.
    """
    from concourse.hw_specs import get_activation_tables
    tabs = get_activation_tables(arch)
    keep = "natural_log_exp_and_others"
    if keep not in tabs:
        return
    for name in tabs:
        if name != keep:
            tabs[name].clear()


def build_nc():
    from contextlib import ExitStack

    nc = bacc.Bacc("TRN2", target_bir_lowering=False, debug=False,
                   num_devices=NCORES)
    _pin_act_table_set(nc.m.arch)

    xT_d = nc.dram_tensor("xT", [128, NTQ, 8, TQ], BF16, kind="ExternalInput")
    wq_d = nc.dram_tensor("wq", [128, 8, 128], BF16, kind="ExternalInput")
    wk_d = nc.dram_tensor("wk", [128, 8, 128], BF16, kind="ExternalInput")
    wv_d = nc.dram_tensor("wv", [128, 8, 128], BF16, kind="ExternalInput")
    wo_d = nc.dram_tensor("wo", [128, C], BF16, kind="ExternalInput")
    masks_d = nc.dram_tensor("masks", [128, 4, TQ], BF16,
                             kind="ExternalInput")
    gq_d = nc.dram_tensor("gq", [128, 1], F32, kind="ExternalInput")
    gk_d = nc.dram_tensor("gk", [128, 1], F32, kind="ExternalInput")
    ones2c_d = nc.dram_tensor("ones2c", [128, 65], BF16,
                              kind="ExternalInput")
    bc_d = nc.dram_tensor("bc", [128, 128], BF16, kind="ExternalInput")
    ident_d = nc.dram_tensor("ident", [128, 128], BF16, kind="ExternalInput")
    zeros_d = nc.dram_tensor("zeros", [128, T], BF16, kind="ExternalInput")
    outT_d = nc.dram_tensor("outT", [128, NTQ, 8, TQ], BF16,
                            kind="ExternalOutput")

    with tile.TileContext(nc) as tc, nc.allow_low_precision("bf16 kernel"):
        with ExitStack() as ctx:
            consts = ctx.enter_context(tc.tile_pool(name="consts", bufs=1))
            acts = ctx.enter_context(tc.tile_pool(name="acts", bufs=1))
            rawp = ctx.enter_context(tc.tile_pool(name="rawp", bufs=2))
            sqp = ctx.enter_context(tc.tile_pool(name="sqp", bufs=2))
            vtp = ctx.enter_context(tc.tile_pool(name="vtp", bufs=2))
            lnp = ctx.enter_context(tc.tile_pool(name="lnp", bufs=2))
            ep = ctx.enter_context(tc.tile_pool(name="ep", bufs=4))
            b2p = ctx.enter_context(tc.tile_pool(name="b2p", bufs=2))
            ctxp = ctx.enter_context(tc.tile_pool(name="ctxp", bufs=2))
            stgp = ctx.enter_context(tc.tile_pool(name="stgp", bufs=3))
            ps_st = ctx.enter_context(
                tc.tile_pool(name="ps_st", bufs=2, space="PSUM"))
            ps_ot = ctx.enter_context(
                tc.tile_pool(name="ps_ot", bufs=1, space="PSUM"))
            ps_sh = ctx.enter_context(
                tc.tile_pool(name="ps_sh", bufs=2, space="PSUM"))

            # ---- consts ----
            wq_sb = consts.tile([128, 8, 128], BF16)
            wk_sb = consts.tile([128, 8, 128], BF16)
            wv_sb = consts.tile([128, 8, 128], BF16)
            wo_sb = consts.tile([128, C], BF16)
            msb = consts.tile([128, 4, TQ], BF16)
            gq_sb = consts.tile([128, 1], F32)
            gk_sb = consts.tile([128, 1], F32)
            ones2c = consts.tile([128, 65], BF16)
            bc_sb = consts.tile([128, 128], BF16)
            eps65 = consts.tile([65, 1], F32)
            ident_sb = consts.tile([128, 128], BF16)

            nc.gpsimd.dma_start(out=wq_sb[:], in_=wq_d[:])
            nc.gpsimd.dma_start(out=wk_sb[:], in_=wk_d[:])
            nc.gpsimd.dma_start(out=gq_sb[:], in_=gq_d[:])
            nc.gpsimd.dma_start(out=gk_sb[:], in_=gk_d[:])
            nc.gpsimd.dma_start(out=ones2c[:], in_=ones2c_d[:])
            nc.gpsimd.dma_start(out=bc_sb[:], in_=bc_d[:])
            nc.gpsimd.dma_start(out=wv_sb[:], in_=wv_d[:])
            nc.gpsimd.dma_start(out=msb[:], in_=masks_d[:])
            nc.gpsimd.dma_start(out=wo_sb[:], in_=wo_d[:])
            if not USE_DMA_TRANSPOSE:
                nc.gpsimd.dma_start(out=ident_sb[:], in_=ident_d[:])
            nc.vector.memset(eps65[:], EPS)

            # ---- persistent activations ----
            # chunk 0 arrives as 8 separate per-ci tiles so each projection
            # matmul depends only on its own piece's DMA; chunks 1-3 arrive
            # as one DMA each (they land during earlier compute)
            x0p = [acts.tile([128, TQ], BF16, name=f"x0p{ci}")
                   for ci in range(8)]
            xcs = [None] + [acts.tile([128, 8, TQ], BF16, name=f"xc{c}")
                            for c in range(1, NTQ)]

            def xpiece(c4, ci):
                return x0p[ci][:] if c4 == 0 else xcs[c4][:, ci, :]

            QTn = acts.tile([128, T], BF16)
            KZ = [acts.tile([128, T], BF16, name=f"KZ{h}")
                  for h in range(HPC)]
            # per key-tile r: [V0(64) | ones | pad(15) | V1(64) | ones | pad]
            # head blocks at 32B-aligned offsets 0/80 for the DMA xbar
            V_sb = acts.tile([128, NTK, 160], BF16)
            sg128 = acts.tile([128, T], BF16)
            rinv = {"q": acts.tile([128, TQ], BF16, name="rinv_q"),
                    "k": acts.tile([128, TQ], BF16, name="rinv_k")}
            sgf = [acts.tile([1, T], F32, name=f"sgf{h}")
                   for h in range(HPC)]
            sgd = [acts.tile([1, T], F32, name=f"sgd{h}")
                   for h in range(HPC)]

            # zero-init (rows outside the written ranges must stay finite);
            # bulk zeroing arrives via DMA so no compute engine pays for it
            vview = V_sb[:].rearrange("p r (a b) -> p r a b", b=80)
            nc.vector.memset(vview[:, :, :, 64:65], 1.0)
            nc.vector.memset(rinv["q"][:], 0.0)
            nc.vector.memset(rinv["k"][:], 0.0)

            # ---- input stream ----
            # chunk-0 pieces alone on the sync queue (they gate the first
            # matmuls); zero-fills and chunks 1-3 ride the otherwise-idle
            # scalar HWDGE queue so neither stream serializes the other
            for ci in range(8):
                nc.sync.dma_start(out=x0p[ci][:], in_=xT_d[:, 0, ci, :])
            nc.scalar.dma_start(out=KZ[0][:], in_=zeros_d[:])
            nc.scalar.dma_start(out=KZ[1][:], in_=zeros_d[:])
            nc.scalar.dma_start(out=xcs[1][:], in_=xT_d[:, 1])
            nc.scalar.dma_start(out=sg128[:], in_=zeros_d[:])
            for c4 in range(2, NTQ):
                nc.scalar.dma_start(out=xcs[c4][:], in_=xT_d[:, c4])

            # ---------- B(c): projections + RMS for chunk c ----------
            # thunk order spreads the serial RMS chains (raw->sq->sums->
            # ln->exp->bb) between independent projection matmuls so the
            # PE queue never head-of-line blocks on a cross-engine dep
            def emit_b(c4):
                sl = slice(TQ * c4, TQ * (c4 + 1))

                def proj_head(w_sb, st):
                    def f():
                        pp = ps_sh.tile([128, TQ], F32, tag="sh", name="pp")
                        st["pp"] = pp
                        for ci in range(4):
                            nc.tensor.matmul(
                                pp[:], w_sb[:, ci, :], xpiece(c4, ci),
                                start=(ci == 0), stop=False)
                    return f

                def proj_tail(w_sb, st):
                    def f():
                        pp = st["pp"]
                        for ci in range(4, 8):
                            nc.tensor.matmul(
                                pp[:], w_sb[:, ci, :], xpiece(c4, ci),
                                start=False, stop=(ci == 7))
                        raw = rawp.tile([128, TQ], BF16, tag="raw",
                                        name="raw")
                        nc.vector.tensor_copy(raw[:], pp[:])
                        sq = sqp.tile([128, TQ], BF16, tag="sq", name="sq")
                        nc.vector.tensor_mul(sq[:], raw[:], raw[:])
                        st["raw"] = raw
                        st["sq"] = sq
                    return f

                def rms(qk, st):
                    def f():
                        sums = ps_sh.tile([65, TQ], F32, tag="sh",
                                          name="sums")
                        nc.tensor.matmul(sums[:], ones2c[:], st["sq"][:],
                                         start=True, stop=True)
                        rln = lnp.tile([65, TQ], F32, tag="ln", name="rln")
                        nc.scalar.activation(rln[:], sums[:], AF.Ln,
                                             bias=eps65[:], scale=1.0 / D)
                        nc.scalar.activation(rinv[qk][0:65, :], rln[:],
                                             AF.Exp, scale=-0.5)
                    return f

                def norm(qk, g_sb, st):
                    def f():
                        bb = ps_sh.tile([128, TQ], F32, tag="sh", name="bb")
                        nc.tensor.matmul(bb[:], bc_sb[:], rinv[qk][:],
                                         start=True, stop=True)
                        raw = st["raw"]
                        if qk == "q":
                            nc.vector.scalar_tensor_tensor(
                                out=QTn[:, sl], in0=raw[:], scalar=g_sb[:],
                                in1=bb[:], op0=MUL, op1=MUL)
                        else:
                            for h in range(HPC):
                                hsl = slice(64 * h, 64 * (h + 1))
                                nc.vector.scalar_tensor_tensor(
                                    out=KZ[h][hsl, sl], in0=raw[hsl, :],
                                    scalar=g_sb[hsl, :], in1=bb[hsl, :],
                                    op0=MUL, op1=MUL)
                    return f

                stq, stk, stv = {}, {}, {}

                def v_head():
                    pv = ps_sh.tile([128, TQ], F32, tag="sh", name="pv")
                    stv["pv"] = pv
                    for ci in range(4):
                        nc.tensor.matmul(
                            pv[:], wv_sb[:, ci, :], xpiece(c4, ci),
                            start=(ci == 0), stop=False)

                def v_tail():
                    pv = stv["pv"]
                    for ci in range(4, 8):
                        nc.tensor.matmul(
                            pv[:], wv_sb[:, ci, :], xpiece(c4, ci),
                            start=False, stop=(ci == 7))
                    vt = vtp.tile([128, TQ], BF16, tag="vt", name="vt")
                    nc.vector.tensor_copy(vt[:], pv[:])
                    stv["vt"] = vt

                def v_tr(rl0):
                    def f():
                        vt = stv["vt"]
                        for rl in (rl0, rl0 + 1):
                            r = 4 * c4 + rl
                            if USE_DMA_TRANSPOSE:
                                for h in range(HPC):
                                    nc.sync.dma_start_transpose(
                                        V_sb[:, r, 80 * h:80 * h + 64],
                                        vt[64 * h:64 * (h + 1),
                                           128 * rl:128 * (rl + 1)])
                            else:
                                tp = ps_sh.tile([128, 128], BF16, tag="sh",
                                                name=f"tp{r}")
                                nc.tensor.transpose(
                                    tp[:], vt[:, 128 * rl:128 * (rl + 1)],
                                    ident_sb[:])
                                dst = V_sb[:, r, :].rearrange(
                                    "p (a b) -> p a b", b=80)[:, :, 0:64]
                                src = tp[:].rearrange("p (a b) -> p a b",
                                                      b=64)
                                nc.vector.tensor_copy(dst, src)
                    return f

                return [proj_head(wq_sb, stq), proj_tail(wq_sb, stq),
                        proj_head(wk_sb, stk), rms("q", stq),
                        proj_tail(wk_sb, stk), norm("q", gq_sb, stq),
                        rms("k", stk), v_head,
                        norm("k", gk_sb, stk), v_tail,
                        v_tr(0), v_tr(2)]

            # ---------- wo(c): normalize + output projection ----------
            def make_prep_h(c4, ot):
                sl = slice(TQ * c4, TQ * (c4 + 1))

                def prep_h(h):
                    # 1/sum_exp for head h, straight off its ot bank; runs
                    # while the other head's groups are still streaming
                    nc.vector.tensor_copy(sgd[h][0:1, sl], ot[h][64:65, :])
                    nc.vector.reciprocal_approx_fast(
                        out=sgf[h][0:1, sl], in_=sgd[h][0:1, sl])
                    nc.vector.tensor_copy(
                        sg128[64 * h:64 * h + 1, sl], sgf[h][0:1, sl])
                return prep_h

            def make_wo_thunks(c4, ot, n_tk):
                sl = slice(TQ * c4, TQ * (c4 + 1))
                st = {}
                thunks = []

                def prep():
                    b2 = ps_sh.tile([128, TQ], F32, tag="sh", name="b2")
                    nc.tensor.matmul(b2[:], bc_sb[:], sg128[:, sl],
                                     start=True, stop=True)
                    b2s = b2p.tile([128, TQ], BF16, tag="b2", name="b2s")
                    nc.vector.tensor_copy(b2s[:], b2[:])
                    ctxT = ctxp.tile([128, TQ], BF16, tag="ctx", name="ctx")
                    for h in range(HPC):
                        hsl = slice(64 * h, 64 * (h + 1))
                        nc.vector.scalar_tensor_tensor(
                            out=ctxT[hsl, :], in0=ot[h][0:64, :], scalar=1.0,
                            in1=b2s[hsl, :], op0=MUL, op1=MUL)
                    st["ctx"] = ctxT

                thunks.append(prep)

                def wo_mu(mu):
                    def f():
                        wop = ps_sh.tile([128, TQ], F32, tag="sh",
                                         name=f"wop{mu}")
                        nc.tensor.matmul(
                            wop[:], wo_sb[:, 128 * mu:128 * (mu + 1)],
                            st["ctx"][:], start=True, stop=True)
                        stg = stgp.tile([128, TQ], BF16, tag="stg",
                                        name=f"stg{mu}")
                        # last chunk's staging is the kernel tail: split it
                        # across ACT (idle after the last exp) and DVE
                        if c4 == NTQ - 1 and mu % 2 == 1:
                            nc.scalar.activation(stg[:], wop[:], AF.Copy)
                        else:
                            nc.vector.tensor_copy(stg[:], wop[:])
                        nc.sync.dma_start(out=outT_d[:, c4, mu, :],
                                          in_=stg[:])
                    return f

                for mu in range(8):
                    thunks.append(wo_mu(mu))
                return thunks

            # ---------- main emission ----------
            for t in emit_b(0):
                t()
            wo_prev = None
            mask_i = 0
            for c4 in range(NTQ):
                sl = slice(TQ * c4, TQ * (c4 + 1))
                n_tk = 4 * (c4 + 1)
                ot = [ps_ot.tile([65, TQ], F32, tag=f"ot{h}",
                                 name=f"ot{h}_{c4}")
                      for h in range(HPC)]

                sched = deque()
                if wo_prev is not None:
                    sched.extend(wo_prev)
                if c4 < NTQ - 1:
                    sched.extend(emit_b(c4 + 1))
                # head-outer: ot[0] completes halfway through the chunk so
                # its denominator prep overlaps head-1's groups
                groups = [(g, h) for h in range(HPC)
                          for g in range(n_tk // 2)]
                per = -(-len(sched) // len(groups))  # ceil division
                prep_h = make_prep_h(c4, ot)

                def emit_pv(g, h, e_t):
                    for j in range(2):
                        r = 2 * g + j
                        nc.tensor.matmul(
                            ot[h][:], V_sb[:, r, 80 * h:80 * h + 65],
                            e_t[:, TQ * j:TQ * (j + 1)],
                            start=(r == 0), stop=(r == n_tk - 1))

                prev = None
                for (g, h) in groups:
                    st_t = ps_st.tile([128, 2 * TQ], F32, tag="st",
                                      name="st")
                    for j in range(2):
                        r = 2 * g + j
                        nc.tensor.matmul(
                            st_t[:, TQ * j:TQ * (j + 1)],
                            KZ[h][:, 128 * r:128 * (r + 1)],
                            QTn[:, sl], start=True, stop=True)
                    e_t = ep.tile([128, 2 * TQ], BF16, tag="e", name="e")
                    nc.scalar.activation(e_t[:], st_t[:], AF.Exp,
                                         scale=SCALE)
                    s0 = 2 * g - 4 * c4
                    if s0 >= 0:
                        ev = e_t[:].rearrange("p (s f) -> p s f", f=TQ)
                        eng = (nc.gpsimd
                               if mask_i % MASK_GP_EVERY == MASK_GP_EVERY - 1
                               else nc.vector)
                        eng.tensor_mul(ev, ev, msb[:, s0:s0 + 2, :])
                        mask_i += 1
                    if prev is not None:
                        emit_pv(*prev)
                        if prev[1] == 0 and h == 1 and g == 0:
                            prep_h(0)
                    prev = (g, h, e_t)
                    for _ in range(per):
                        if sched:
                            sched.popleft()()
                emit_pv(*prev)
                prep_h(1)
                while sched:
                    sched.popleft()()
                wo_prev = make_wo_thunks(c4, ot, n_tk)
            for t in wo_prev:
                t()

    nc.compile()
    return nc


_NC_CACHE = None


def _get_nc():
    global _NC_CACHE
    if _NC_CACHE is None:
        _NC_CACHE = build_nc()
    return _NC_CACHE


def _make_in_maps(x, w_q, w_k, w_v, w_o, q_gamma, k_gamma):
    x = np.asarray(x, dtype=np.float32).reshape(T, C)
    # xT[p, c4, ci, t'] = x[512*c4 + t', 128*ci + p]
    xT = np.ascontiguousarray(
        x.reshape(NTQ, TQ, 8, 128).transpose(3, 0, 2, 1)).astype(NP_BF16)

    p = np.arange(128)
    f = np.arange(TQ)
    masks = np.zeros((128, 4, TQ), dtype=NP_BF16)
    for s in range(4):
        masks[:, s, :] = (f[None, :] >= (p[:, None] + 128 * s)).astype(
            NP_BF16)

    gq = np.tile(np.asarray(q_gamma, np.float32), 2).reshape(128, 1)
    gk = np.tile(np.asarray(k_gamma, np.float32), 2).reshape(128, 1)
    ones2c = np.zeros((128, 65), dtype=NP_BF16)
    ones2c[0:64, 0] = 1
    ones2c[64:128, 64] = 1
    # broadcast stationary: row 0 -> out partitions 0..63,
    # row 64 -> out partitions 64..127
    bc = np.zeros((128, 128), dtype=NP_BF16)
    bc[0, 0:64] = 1
    bc[64, 64:128] = 1
    ident = np.eye(128, dtype=NP_BF16)

    common = dict(xT=xT, masks=masks, gq=gq, gk=gk, ones2c=ones2c, bc=bc,
                  ident=ident, zeros=np.zeros((128, T), dtype=NP_BF16))

    in_maps = []
    for i in range(NCORES):
        rows = slice(JPC * i, JPC * (i + 1))

        def wsw(w):
            # [p, ci, j] = W[rows][j, ci*128+p]
            W = np.asarray(w, np.float32)[rows]           # [128, C]
            return np.ascontiguousarray(
                W.reshape(128, 8, 128).transpose(2, 1, 0)).astype(NP_BF16)

        wo = np.asarray(w_o, np.float32)[:, rows].T        # [128, C]
        in_maps.append(dict(common, wq=wsw(w_q), wk=wsw(w_k), wv=wsw(w_v),
                            wo=np.ascontiguousarray(wo).astype(NP_BF16)))
    return in_maps


def _run(x, w_q, w_k, w_v, w_o, q_gamma, k_gamma, trace=False):
    import time

    from concourse.bass_utils import run_bass_kernel_spmd
    nc = _get_nc()
    in_maps = _make_in_maps(x, w_q, w_k, w_v, w_o, q_gamma, k_gamma)
    res = None
    for attempt in range(3):
        try:
            res = run_bass_kernel_spmd(nc, in_maps, list(range(NCORES)),
                                       trace=trace)
            break
        except Exception:
            # rare transient NRT_EXEC_UNIT_UNRECOVERABLE under axon; the
            # terminal resets the device on the next load
            if attempt == 2:
                raise
            time.sleep(3.0)
    acc = np.zeros((128, NTQ, 8, TQ), dtype=np.float64)
    for r in res.results:
        acc += r["outT"].astype(np.float64)
    # out[512*c4 + t', 128*mu + p] = acc[p, c4, mu, t']
    out = acc.transpose(1, 3, 2, 0).reshape(T, C).astype(np.float32)
    return out.reshape(1, T, C), res


def kernel(x, w_q, w_k, w_v, w_o, q_gamma, k_gamma):
    out, _ = _run(x, w_q, w_k, w_v, w_o, q_gamma, k_gamma, trace=False)
    return out


# revision 33
# speedup vs baseline: 1.0989x; 1.0989x over previous
"""Trainium2 Bass kernel for nn_Attention_64819646431478.

Single-layer causal attention, B=1, T=2048, DIM=1024, 16 heads, head_dim=64,
f32, with RMSNorm (eps=f32 eps) on Q and K heads.

Sharding: tensor-parallel over heads across 8 NeuronCores (2 heads/core).
Each core computes its heads' Q/K/V projections, causal attention, and the
partial output projection against its 128-row slice of w_o; the host sums
the 8 partial outputs (the "all-reduce" of the hint, done at gather time).

v3 design notes (vs the v2 baseline at 132us):
  - ONE ACT table set for the whole kernel (natural_log_exp_and_others):
    softmax exp, plus RMS rsqrt computed as exp(-0.5*ln(ms+eps)).  No
    mid-kernel ACT_TABLE_LOADs, so no PE stall at the B->C boundary.
  - Chunk-interleaved emission: projections for chunk c+1 and the output
    projection for chunk c-1 are emitted *between* the attention groups of
    chunk c, so the PE queue always has dependency-free matmuls to run
    while ACT streams the exp of the current groups.  This keeps the PE
    HAM-warm (2.4 GHz) end to end.
  - V transpose moved off the PE/DVE onto the DMA xbar transpose engine.
  - DVE diet: squares from the bf16 raw copy (tensor_mul), rinv/sg128 rows
    written in place (no rec/sgf shuffling), big memsets on gpsimd, every
    4th causal-mask multiply on gpsimd.
  - xT is chunk-major in dram; chunk 0 arrives as 8 per-ci pieces so the
    first projection matmul starts ~1us in.
  - PSUM: st 3x[128,512] + ot 2 + shared(pp/sums/bb/wo/b2) 3 = 8 banks,
    sized so the interleaved B/C/wo streams coexist.
"""

import os
import sys
import types
from collections import deque

import numpy as np

# --- environment bootstrap (harness may run us from a bare directory) ---
for _p in ("/root/.axon_site", "/root/.axon_site/_ro/trn_rl_repo",
           "/root/.axon_site/_ro/pypackages", "/opt/trn_rl_repo"):
    if os.path.isdir(_p) and _p not in sys.path:
        sys.path.append(_p)


def _install_ntff_shim():
    """Provide antenv.axon_hooks (missing in this image) so trace=True works."""
    if "antenv.axon_hooks" in sys.modules:
        return
    mod = types.ModuleType("antenv.axon_hooks")
    mod._hook = None
    mod.set_axon_ntff_profile_hook = lambda h: setattr(mod, "_hook", h)
    mod.get_axon_ntff_profile_hook = lambda: mod._hook
    sys.modules["antenv.axon_hooks"] = mod
    try:
        import antenv
        antenv.axon_hooks = mod
        from trn_agent_boot.trn_boot import _ntff_profile_via_ctypes
        mod.set_axon_ntff_profile_hook(
            _ntff_profile_via_ctypes("/opt/axon/libaxon_pjrt.so"))
    except Exception:
        pass


_install_ntff_shim()

import ml_dtypes  # noqa: E402

import concourse.mybir as mybir  # noqa: E402
import concourse.tile as tile  # noqa: E402
from concourse import bacc  # noqa: E402

F32 = mybir.dt.float32
BF16 = mybir.dt.bfloat16
NP_BF16 = ml_dtypes.bfloat16
AF = mybir.ActivationFunctionType
MUL = mybir.AluOpType.mult

T = 2048
C = 1024
D = 64
NCORES = 8
HPC = 2            # heads per core
JPC = HPC * D      # 128 j-columns per core
NTQ = 4            # tq chunks of 512
TQ = 512
NTK = 16           # tk tiles of 128
EPS = float(np.finfo(np.float32).eps)
SCALE = float(D) ** -0.5

USE_DMA_TRANSPOSE = False  # V transpose on DMA xbar instead of PE
MASK_GP_EVERY = 10 ** 9    # every Nth mask multiply goes to gpsimd
STG_ACT_EVERY = 4          # every Nth wo staging copy goes to ACT


def _pin_act_table_set(arch):
    """Steer the ACT table-set chooser to natural_log_exp_and_others.

    The kernel only uses Exp, Ln and Copy on the scalar engine, and one set
    contains all three.  The default chooser binds Exp to exp_and_others and
    Ln to natural_log, which thrashes ~2.7us of ACT_TABLE_LOAD per RMS
    instance.  Emptying the *contents* of the competing cached set entries
    (order and indices unchanged, so the runtime set ids stay correct) makes
    the fixpoint pass pick the combined set everywhere -> a single load.
    """
    from concourse.hw_specs import get_activation_tables
    tabs = get_activation_tables(arch)
    keep = "natural_log_exp_and_others"
    if keep not in tabs:
        return
    for name in tabs:
        if name != keep:
            tabs[name].clear()


def build_nc():
    from contextlib import ExitStack

    nc = bacc.Bacc("TRN2", target_bir_lowering=False, debug=False,
                   num_devices=NCORES)
    _pin_act_table_set(nc.m.arch)

    xT_d = nc.dram_tensor("xT", [128, NTQ, 8, TQ], BF16, kind="ExternalInput")
    wq_d = nc.dram_tensor("wq", [128, 8, 128], BF16, kind="ExternalInput")
    wk_d = nc.dram_tensor("wk", [128, 8, 128], BF16, kind="ExternalInput")
    wv_d = nc.dram_tensor("wv", [128, 8, 128], BF16, kind="ExternalInput")
    wo_d = nc.dram_tensor("wo", [128, C], BF16, kind="ExternalInput")
    masks_d = nc.dram_tensor("masks", [128, 4, TQ], BF16,
                             kind="ExternalInput")
    gq_d = nc.dram_tensor("gq", [128, 1], F32, kind="ExternalInput")
    gk_d = nc.dram_tensor("gk", [128, 1], F32, kind="ExternalInput")
    ones2c_d = nc.dram_tensor("ones2c", [128, 65], BF16,
                              kind="ExternalInput")
    bc_d = nc.dram_tensor("bc", [128, 128], BF16, kind="ExternalInput")
    ident_d = nc.dram_tensor("ident", [128, 128], BF16, kind="ExternalInput")
    zeros_d = nc.dram_tensor("zeros", [128, T], BF16, kind="ExternalInput")
    outT_d = nc.dram_tensor("outT", [128, NTQ, 8, TQ], BF16,
                            kind="ExternalOutput")

    with tile.TileContext(nc) as tc, nc.allow_low_precision("bf16 kernel"):
        with ExitStack() as ctx:
            consts = ctx.enter_context(tc.tile_pool(name="consts", bufs=1))
            acts = ctx.enter_context(tc.tile_pool(name="acts", bufs=1))
            rawp = ctx.enter_context(tc.tile_pool(name="rawp", bufs=2))
            sqp = ctx.enter_context(tc.tile_pool(name="sqp", bufs=2))
            vtp = ctx.enter_context(tc.tile_pool(name="vtp", bufs=2))
            lnp = ctx.enter_context(tc.tile_pool(name="lnp", bufs=2))
            ep = ctx.enter_context(tc.tile_pool(name="ep", bufs=4))
            b2p = ctx.enter_context(tc.tile_pool(name="b2p", bufs=2))
            ctxp = ctx.enter_context(tc.tile_pool(name="ctxp", bufs=2))
            stgp = ctx.enter_context(tc.tile_pool(name="stgp", bufs=3))
            ps_st = ctx.enter_context(
                tc.tile_pool(name="ps_st", bufs=2, space="PSUM"))
            ps_ot = ctx.enter_context(
                tc.tile_pool(name="ps_ot", bufs=1, space="PSUM"))
            ps_sh = ctx.enter_context(
                tc.tile_pool(name="ps_sh", bufs=2, space="PSUM"))

            # ---- consts ----
            wq_sb = consts.tile([128, 8, 128], BF16)
            wk_sb = consts.tile([128, 8, 128], BF16)
            wv_sb = consts.tile([128, 8, 128], BF16)
            wo_sb = consts.tile([128, C], BF16)
            msb = consts.tile([128, 4, TQ], BF16)
            gq_sb = consts.tile([128, 1], F32)
            gk_sb = consts.tile([128, 1], F32)
            ones2c = consts.tile([128, 65], BF16)
            bc_sb = consts.tile([128, 128], BF16)
            eps65 = consts.tile([65, 1], F32)
            ident_sb = consts.tile([128, 128], BF16)

            nc.gpsimd.dma_start(out=wq_sb[:], in_=wq_d[:])
            nc.gpsimd.dma_start(out=wk_sb[:], in_=wk_d[:])
            nc.gpsimd.dma_start(out=gq_sb[:], in_=gq_d[:])
            nc.gpsimd.dma_start(out=gk_sb[:], in_=gk_d[:])
            nc.gpsimd.dma_start(out=ones2c[:], in_=ones2c_d[:])
            nc.gpsimd.dma_start(out=bc_sb[:], in_=bc_d[:])
            nc.gpsimd.dma_start(out=wv_sb[:], in_=wv_d[:])
            nc.gpsimd.dma_start(out=msb[:], in_=masks_d[:])
            nc.gpsimd.dma_start(out=wo_sb[:], in_=wo_d[:])
            if not USE_DMA_TRANSPOSE:
                nc.gpsimd.dma_start(out=ident_sb[:], in_=ident_d[:])
            nc.vector.memset(eps65[:], EPS)

            # ---- persistent activations ----
            # chunk 0 arrives as 8 separate per-ci tiles so each projection
            # matmul depends only on its own piece's DMA; chunks 1-3 arrive
            # as one DMA each (they land during earlier compute)
            x0p = [acts.tile([128, TQ], BF16, name=f"x0p{ci}")
                   for ci in range(8)]
            xcs = [None] + [acts.tile([128, 8, TQ], BF16, name=f"xc{c}")
                            for c in range(1, NTQ)]

            def xpiece(c4, ci):
                return x0p[ci][:] if c4 == 0 else xcs[c4][:, ci, :]

            QTn = acts.tile([128, T], BF16)
            KZ = [acts.tile([128, T], BF16, name=f"KZ{h}")
                  for h in range(HPC)]
            # per key-tile r: [V0(64) | ones | pad(15) | V1(64) | ones | pad]
            # head blocks at 32B-aligned offsets 0/80 for the DMA xbar
            V_sb = acts.tile([128, NTK, 160], BF16)
            sg128 = acts.tile([128, T], BF16)
            rinv = {"q": acts.tile([128, TQ], BF16, name="rinv_q"),
                    "k": acts.tile([128, TQ], BF16, name="rinv_k")}
            sgf = [acts.tile([1, T], F32, name=f"sgf{h}")
                   for h in range(HPC)]
            sgd = [acts.tile([1, T], F32, name=f"sgd{h}")
                   for h in range(HPC)]

            # zero-init (rows outside the written ranges must stay finite);
            # bulk zeroing arrives via DMA so no compute engine pays for it
            vview = V_sb[:].rearrange("p r (a b) -> p r a b", b=80)
            nc.vector.memset(vview[:, :, :, 64:65], 1.0)
            nc.vector.memset(rinv["q"][:], 0.0)
            nc.vector.memset(rinv["k"][:], 0.0)

            # ---- input stream ----
            # chunk-0 pieces split across both HWDGE queues (double arrival
            # rate -> denser early matmuls -> HAM warms sooner); zero-fills
            # and chunks 1-3 follow on the scalar queue
            for ci in range(8):
                eng = nc.sync if ci % 2 == 0 else nc.scalar
                eng.dma_start(out=x0p[ci][:], in_=xT_d[:, 0, ci, :])
            nc.scalar.dma_start(out=KZ[0][:], in_=zeros_d[:])
            nc.scalar.dma_start(out=KZ[1][:], in_=zeros_d[:])
            nc.scalar.dma_start(out=xcs[1][:], in_=xT_d[:, 1])
            nc.scalar.dma_start(out=sg128[:], in_=zeros_d[:])
            for c4 in range(2, NTQ):
                nc.scalar.dma_start(out=xcs[c4][:], in_=xT_d[:, c4])

            # ---------- B(c): projections + RMS for chunk c ----------
            # thunk order spreads the serial RMS chains (raw->sq->sums->
            # ln->exp->bb) between independent projection matmuls so the
            # PE queue never head-of-line blocks on a cross-engine dep
            def emit_b(c4):
                sl = slice(TQ * c4, TQ * (c4 + 1))

                def proj_head(w_sb, st):
                    def f():
                        pp = ps_sh.tile([128, TQ], F32, tag="sh", name="pp")
                        st["pp"] = pp
                        for ci in range(4):
                            nc.tensor.matmul(
                                pp[:], w_sb[:, ci, :], xpiece(c4, ci),
                                start=(ci == 0), stop=False)
                    return f

                def proj_tail(w_sb, st):
                    def f():
                        pp = st["pp"]
                        for ci in range(4, 8):
                            nc.tensor.matmul(
                                pp[:], w_sb[:, ci, :], xpiece(c4, ci),
                                start=False, stop=(ci == 7))
                        raw = rawp.tile([128, TQ], BF16, tag="raw",
                                        name="raw")
                        nc.vector.tensor_copy(raw[:], pp[:])
                        sq = sqp.tile([128, TQ], BF16, tag="sq", name="sq")
                        nc.vector.tensor_mul(sq[:], raw[:], raw[:])
                        st["raw"] = raw
                        st["sq"] = sq
                    return f

                def rms(qk, st):
                    def f():
                        sums = ps_sh.tile([65, TQ], F32, tag="sh",
                                          name="sums")
                        nc.tensor.matmul(sums[:], ones2c[:], st["sq"][:],
                                         start=True, stop=True)
                        rln = lnp.tile([65, TQ], F32, tag="ln", name="rln")
                        nc.scalar.activation(rln[:], sums[:], AF.Ln,
                                             bias=eps65[:], scale=1.0 / D)
                        nc.scalar.activation(rinv[qk][0:65, :], rln[:],
                                             AF.Exp, scale=-0.5)
                    return f

                def norm(qk, g_sb, st):
                    def f():
                        bb = ps_sh.tile([128, TQ], F32, tag="sh", name="bb")
                        nc.tensor.matmul(bb[:], bc_sb[:], rinv[qk][:],
                                         start=True, stop=True)
                        raw = st["raw"]
                        if qk == "q":
                            nc.vector.scalar_tensor_tensor(
                                out=QTn[:, sl], in0=raw[:], scalar=g_sb[:],
                                in1=bb[:], op0=MUL, op1=MUL)
                        else:
                            for h in range(HPC):
                                hsl = slice(64 * h, 64 * (h + 1))
                                nc.vector.scalar_tensor_tensor(
                                    out=KZ[h][hsl, sl], in0=raw[hsl, :],
                                    scalar=g_sb[hsl, :], in1=bb[hsl, :],
                                    op0=MUL, op1=MUL)
                    return f

                stq, stk, stv = {}, {}, {}

                def v_head():
                    pv = ps_sh.tile([128, TQ], F32, tag="sh", name="pv")
                    stv["pv"] = pv
                    for ci in range(4):
                        nc.tensor.matmul(
                            pv[:], wv_sb[:, ci, :], xpiece(c4, ci),
                            start=(ci == 0), stop=False)

                def v_tail():
                    pv = stv["pv"]
                    for ci in range(4, 8):
                        nc.tensor.matmul(
                            pv[:], wv_sb[:, ci, :], xpiece(c4, ci),
                            start=False, stop=(ci == 7))
                    vt = vtp.tile([128, TQ], BF16, tag="vt", name="vt")
                    nc.vector.tensor_copy(vt[:], pv[:])
                    stv["vt"] = vt

                def v_tr(rl0):
                    def f():
                        vt = stv["vt"]
                        for rl in (rl0, rl0 + 1):
                            r = 4 * c4 + rl
                            if USE_DMA_TRANSPOSE:
                                for h in range(HPC):
                                    nc.sync.dma_start_transpose(
                                        V_sb[:, r, 80 * h:80 * h + 64],
                                        vt[64 * h:64 * (h + 1),
                                           128 * rl:128 * (rl + 1)])
                            else:
                                tp = ps_sh.tile([128, 128], BF16, tag="sh",
                                                name=f"tp{r}")
                                nc.tensor.transpose(
                                    tp[:], vt[:, 128 * rl:128 * (rl + 1)],
                                    ident_sb[:])
                                dst = V_sb[:, r, :].rearrange(
                                    "p (a b) -> p a b", b=80)[:, :, 0:64]
                                src = tp[:].rearrange("p (a b) -> p a b",
                                                      b=64)
                                nc.vector.tensor_copy(dst, src)
                    return f

                return [proj_head(wq_sb, stq), proj_tail(wq_sb, stq),
                        proj_head(wk_sb, stk), rms("q", stq),
                        proj_tail(wk_sb, stk), norm("q", gq_sb, stq),
                        rms("k", stk), v_head,
                        norm("k", gk_sb, stk), v_tail,
                        v_tr(0), v_tr(2)]

            # ---------- wo(c): normalize + output projection ----------
            def make_prep_h(c4, ot):
                sl = slice(TQ * c4, TQ * (c4 + 1))

                def prep_h(h):
                    # 1/sum_exp for head h, straight off its ot bank; runs
                    # while the other head's groups are still streaming
                    nc.vector.tensor_copy(sgd[h][0:1, sl], ot[h][64:65, :])
                    nc.vector.reciprocal_approx_fast(
                        out=sgf[h][0:1, sl], in_=sgd[h][0:1, sl])
                    nc.vector.tensor_copy(
                        sg128[64 * h:64 * h + 1, sl], sgf[h][0:1, sl])
                return prep_h

            def make_wo_thunks(c4, ot, n_tk):
                sl = slice(TQ * c4, TQ * (c4 + 1))
                st = {}
                thunks = []

                def prep():
                    b2 = ps_sh.tile([128, TQ], F32, tag="sh", name="b2")
                    nc.tensor.matmul(b2[:], bc_sb[:], sg128[:, sl],
                                     start=True, stop=True)
                    b2s = b2p.tile([128, TQ], BF16, tag="b2", name="b2s")
                    nc.vector.tensor_copy(b2s[:], b2[:])
                    ctxT = ctxp.tile([128, TQ], BF16, tag="ctx", name="ctx")
                    for h in range(HPC):
                        hsl = slice(64 * h, 64 * (h + 1))
                        nc.vector.scalar_tensor_tensor(
                            out=ctxT[hsl, :], in0=ot[h][0:64, :], scalar=1.0,
                            in1=b2s[hsl, :], op0=MUL, op1=MUL)
                    st["ctx"] = ctxT

                thunks.append(prep)

                def wo_mu(mu):
                    def f():
                        wop = ps_sh.tile([128, TQ], F32, tag="sh",
                                         name=f"wop{mu}")
                        nc.tensor.matmul(
                            wop[:], wo_sb[:, 128 * mu:128 * (mu + 1)],
                            st["ctx"][:], start=True, stop=True)
                        stg = stgp.tile([128, TQ], BF16, tag="stg",
                                        name=f"stg{mu}")
                        # last chunk's staging is the kernel tail: split it
                        # across ACT (idle after the last exp) and DVE
                        if c4 == NTQ - 1 and mu % 2 == 1:
                            nc.scalar.activation(stg[:], wop[:], AF.Copy)
                        else:
                            nc.vector.tensor_copy(stg[:], wop[:])
                        nc.sync.dma_start(out=outT_d[:, c4, mu, :],
                                          in_=stg[:])
                    return f

                for mu in range(8):
                    thunks.append(wo_mu(mu))
                return thunks

            # ---------- main emission ----------
            for t in emit_b(0):
                t()
            wo_prev = None
            mask_i = 0
            for c4 in range(NTQ):
                sl = slice(TQ * c4, TQ * (c4 + 1))
                n_tk = 4 * (c4 + 1)
                ot = [ps_ot.tile([65, TQ], F32, tag=f"ot{h}",
                                 name=f"ot{h}_{c4}")
                      for h in range(HPC)]

                sched = deque()
                if wo_prev is not None:
                    sched.extend(wo_prev)
                if c4 < NTQ - 1:
                    sched.extend(emit_b(c4 + 1))
                # head-outer: ot[0] completes halfway through the chunk so
                # its denominator prep overlaps head-1's groups
                groups = [(g, h) for h in range(HPC)
                          for g in range(n_tk // 2)]
                per = -(-len(sched) // len(groups))  # ceil division
                prep_h = make_prep_h(c4, ot)

                def emit_pv(g, h, e_t):
                    for j in range(2):
                        r = 2 * g + j
                        nc.tensor.matmul(
                            ot[h][:], V_sb[:, r, 80 * h:80 * h + 65],
                            e_t[:, TQ * j:TQ * (j + 1)],
                            start=(r == 0), stop=(r == n_tk - 1))

                prev = None
                for (g, h) in groups:
                    st_t = ps_st.tile([128, 2 * TQ], F32, tag="st",
                                      name="st")
                    for j in range(2):
                        r = 2 * g + j
                        nc.tensor.matmul(
                            st_t[:, TQ * j:TQ * (j + 1)],
                            KZ[h][:, 128 * r:128 * (r + 1)],
                            QTn[:, sl], start=True, stop=True)
                    e_t = ep.tile([128, 2 * TQ], BF16, tag="e", name="e")
                    nc.scalar.activation(e_t[:], st_t[:], AF.Exp,
                                         scale=SCALE)
                    s0 = 2 * g - 4 * c4
                    if s0 >= 0:
                        ev = e_t[:].rearrange("p (s f) -> p s f", f=TQ)
                        eng = (nc.gpsimd
                               if mask_i % MASK_GP_EVERY == MASK_GP_EVERY - 1
                               else nc.vector)
                        eng.tensor_mul(ev, ev, msb[:, s0:s0 + 2, :])
                        mask_i += 1
                    if prev is not None:
                        emit_pv(*prev)
                        if prev[1] == 0 and h == 1 and g == 0:
                            # queue head-0's denominator prep; it lands a
                            # few groups later so it never preempts the
                            # mask multiplies in the DVE FIFO
                            sched.append(lambda: prep_h(0))
                    prev = (g, h, e_t)
                    for _ in range(per):
                        if sched:
                            sched.popleft()()
                emit_pv(*prev)
                prep_h(1)
                while sched:
                    sched.popleft()()
                wo_prev = make_wo_thunks(c4, ot, n_tk)
            for t in wo_prev:
                t()

    nc.compile()
    return nc


_NC_CACHE = None


def _get_nc():
    global _NC_CACHE
    if _NC_CACHE is None:
        _NC_CACHE = build_nc()
    return _NC_CACHE


def _make_in_maps(x, w_q, w_k, w_v, w_o, q_gamma, k_gamma):
    x = np.asarray(x, dtype=np.float32).reshape(T, C)
    # xT[p, c4, ci, t'] = x[512*c4 + t', 128*ci + p]
    xT = np.ascontiguousarray(
        x.reshape(NTQ, TQ, 8, 128).transpose(3, 0, 2, 1)).astype(NP_BF16)

    p = np.arange(128)
    f = np.arange(TQ)
    masks = np.zeros((128, 4, TQ), dtype=NP_BF16)
    for s in range(4):
        masks[:, s, :] = (f[None, :] >= (p[:, None] + 128 * s)).astype(
            NP_BF16)

    gq = np.tile(np.asarray(q_gamma, np.float32), 2).reshape(128, 1)
    gk = np.tile(np.asarray(k_gamma, np.float32), 2).reshape(128, 1)
    ones2c = np.zeros((128, 65), dtype=NP_BF16)
    ones2c[0:64, 0] = 1
    ones2c[64:128, 64] = 1
    # broadcast stationary: row 0 -> out partitions 0..63,
    # row 64 -> out partitions 64..127
    bc = np.zeros((128, 128), dtype=NP_BF16)
    bc[0, 0:64] = 1
    bc[64, 64:128] = 1
    ident = np.eye(128, dtype=NP_BF16)

    common = dict(xT=xT, masks=masks, gq=gq, gk=gk, ones2c=ones2c, bc=bc,
                  ident=ident, zeros=np.zeros((128, T), dtype=NP_BF16))

    in_maps = []
    for i in range(NCORES):
        rows = slice(JPC * i, JPC * (i + 1))

        def wsw(w):
            # [p, ci, j] = W[rows][j, ci*128+p]
            W = np.asarray(w, np.float32)[rows]           # [128, C]
            return np.ascontiguousarray(
                W.reshape(128, 8, 128).transpose(2, 1, 0)).astype(NP_BF16)

        wo = np.asarray(w_o, np.float32)[:, rows].T        # [128, C]
        in_maps.append(dict(common, wq=wsw(w_q), wk=wsw(w_k), wv=wsw(w_v),
                            wo=np.ascontiguousarray(wo).astype(NP_BF16)))
    return in_maps


def _run(x, w_q, w_k, w_v, w_o, q_gamma, k_gamma, trace=False):
    import time

    from concourse.bass_utils import run_bass_kernel_spmd
    nc = _get_nc()
    in_maps = _make_in_maps(x, w_q, w_k, w_v, w_o, q_gamma, k_gamma)
    res = None
    for attempt in range(3):
        try:
            res = run_bass_kernel_spmd(nc, in_maps, list(range(NCORES)),
                                       trace=trace)
            break
        except Exception:
            # rare transient NRT_EXEC_UNIT_UNRECOVERABLE under axon; the
            # terminal resets the device on the next load
            if attempt == 2:
                raise
            time.sleep(3.0)
    acc = np.zeros((128, NTQ, 8, TQ), dtype=np.float64)
    for r in res.results:
        acc += r["outT"].astype(np.float64)
    # out[512*c4 + t', 128*mu + p] = acc[p, c4, mu, t']
    out = acc.transpose(1, 3, 2, 0).reshape(T, C).astype(np.float32)
    return out.reshape(1, T, C), res


def kernel(x, w_q, w_k, w_v, w_o, q_gamma, k_gamma):
    out, _ = _run(x, w_q, w_k, w_v, w_o, q_gamma, k_gamma, trace=False)
    return out


# revision 44
# speedup vs baseline: 1.2326x; 1.1216x over previous
"""Trainium2 Bass kernel for nn_Attention_64819646431478.

Single-layer causal attention, B=1, T=2048, DIM=1024, 16 heads, head_dim=64,
f32, with RMSNorm (eps=f32 eps) on Q and K heads.

Sharding: tensor-parallel over heads across 8 NeuronCores (2 heads/core).
Each core computes its heads' Q/K/V projections, causal attention, and the
partial output projection against its 128-row slice of w_o; the host sums
the 8 partial outputs (the "all-reduce" of the hint, done at gather time).

v3 design notes (vs the v2 baseline at 132us):
  - ONE ACT table set for the whole kernel (natural_log_exp_and_others):
    softmax exp, plus RMS rsqrt computed as exp(-0.5*ln(ms+eps)).  No
    mid-kernel ACT_TABLE_LOADs, so no PE stall at the B->C boundary.
  - Chunk-interleaved emission: projections for chunk c+1 and the output
    projection for chunk c-1 are emitted *between* the attention groups of
    chunk c, so the PE queue always has dependency-free matmuls to run
    while ACT streams the exp of the current groups.  This keeps the PE
    HAM-warm (2.4 GHz) end to end.
  - V transpose moved off the PE/DVE onto the DMA xbar transpose engine.
  - DVE diet: squares from the bf16 raw copy (tensor_mul), rinv/sg128 rows
    written in place (no rec/sgf shuffling), big memsets on gpsimd, every
    4th causal-mask multiply on gpsimd.
  - xT is chunk-major in dram; chunk 0 arrives as 8 per-ci pieces so the
    first projection matmul starts ~1us in.
  - PSUM: st 3x[128,512] + ot 2 + shared(pp/sums/bb/wo/b2) 3 = 8 banks,
    sized so the interleaved B/C/wo streams coexist.
"""

import os
import sys
import types
from collections import deque

import numpy as np

# --- environment bootstrap (harness may run us from a bare directory) ---
for _p in ("/root/.axon_site", "/root/.axon_site/_ro/trn_rl_repo",
           "/root/.axon_site/_ro/pypackages", "/opt/trn_rl_repo"):
    if os.path.isdir(_p) and _p not in sys.path:
        sys.path.append(_p)


def _install_ntff_shim():
    """Provide antenv.axon_hooks (missing in this image) so trace=True works."""
    if "antenv.axon_hooks" in sys.modules:
        return
    mod = types.ModuleType("antenv.axon_hooks")
    mod._hook = None
    mod.set_axon_ntff_profile_hook = lambda h: setattr(mod, "_hook", h)
    mod.get_axon_ntff_profile_hook = lambda: mod._hook
    sys.modules["antenv.axon_hooks"] = mod
    try:
        import antenv
        antenv.axon_hooks = mod
        from trn_agent_boot.trn_boot import _ntff_profile_via_ctypes
        mod.set_axon_ntff_profile_hook(
            _ntff_profile_via_ctypes("/opt/axon/libaxon_pjrt.so"))
    except Exception:
        pass


_install_ntff_shim()

import ml_dtypes  # noqa: E402

import concourse.mybir as mybir  # noqa: E402
import concourse.tile as tile  # noqa: E402
from concourse import bacc  # noqa: E402

F32 = mybir.dt.float32
BF16 = mybir.dt.bfloat16
FP8 = mybir.dt.float8e4
NP_BF16 = ml_dtypes.bfloat16
NP_FP8 = ml_dtypes.float8_e4m3fn
AF = mybir.ActivationFunctionType
MUL = mybir.AluOpType.mult
DR = mybir.MatmulPerfMode.DoubleRow

T = 2048
C = 1024
D = 64
NCORES = 8
HPC = 2            # heads per core
JPC = HPC * D      # 128 j-columns per core
NTQ = 4            # tq chunks of 512
TQ = 512
NTK = 16           # tk tiles of 128
EPS = float(np.finfo(np.float32).eps)
SCALE = float(D) ** -0.5

USE_DMA_TRANSPOSE = False  # V transpose on DMA xbar instead of PE
MASK_GP_EVERY = 10 ** 9    # every Nth mask multiply goes to gpsimd
STG_ACT_EVERY = 4          # every Nth wo staging copy goes to ACT


def _pin_act_table_set(arch):
    """Steer the ACT table-set chooser to natural_log_exp_and_others.

    The kernel only uses Exp, Ln and Copy on the scalar engine, and one set
    contains all three.  The default chooser binds Exp to exp_and_others and
    Ln to natural_log, which thrashes ~2.7us of ACT_TABLE_LOAD per RMS
    instance.  Emptying the *contents* of the competing cached set entries
    (order and indices unchanged, so the runtime set ids stay correct) makes
    the fixpoint pass pick the combined set everywhere -> a single load.
    """
    from concourse.hw_specs import get_activation_tables
    tabs = get_activation_tables(arch)
    keep = "natural_log_exp_and_others"
    if keep not in tabs:
        return
    for name in tabs:
        if name != keep:
            tabs[name].clear()


def build_nc():
    from contextlib import ExitStack

    nc = bacc.Bacc("TRN2", target_bir_lowering=False, debug=False,
                   num_devices=NCORES)
    _pin_act_table_set(nc.m.arch)

    xT_d = nc.dram_tensor("xT", [128, NTQ, 8, TQ], BF16, kind="ExternalInput")
    wq_d = nc.dram_tensor("wq", [128, 8, 128], BF16, kind="ExternalInput")
    wk_d = nc.dram_tensor("wk", [128, 8, 128], BF16, kind="ExternalInput")
    wv_d = nc.dram_tensor("wv", [128, 8, 128], BF16, kind="ExternalInput")
    wo_d = nc.dram_tensor("wo", [128, C], BF16, kind="ExternalInput")
    masks_d = nc.dram_tensor("masks", [128, 4, TQ], BF16,
                             kind="ExternalInput")
    gq_d = nc.dram_tensor("gq", [128, 1], F32, kind="ExternalInput")
    gk_d = nc.dram_tensor("gk", [128, 1], F32, kind="ExternalInput")
    ones2c_d = nc.dram_tensor("ones2c", [128, 65], BF16,
                              kind="ExternalInput")
    bc_d = nc.dram_tensor("bc", [128, 128], BF16, kind="ExternalInput")
    ident_d = nc.dram_tensor("ident", [128, 128], BF16, kind="ExternalInput")
    zeros_d = nc.dram_tensor("zeros", [128, T], BF16, kind="ExternalInput")
    outT_d = nc.dram_tensor("outT", [128, NTQ, 8, TQ], BF16,
                            kind="ExternalOutput")

    with tile.TileContext(nc) as tc, nc.allow_low_precision("bf16 kernel"):
        with ExitStack() as ctx:
            consts = ctx.enter_context(tc.tile_pool(name="consts", bufs=1))
            acts = ctx.enter_context(tc.tile_pool(name="acts", bufs=1))
            rawp = ctx.enter_context(tc.tile_pool(name="rawp", bufs=2))
            sqp = ctx.enter_context(tc.tile_pool(name="sqp", bufs=2))
            vtp = ctx.enter_context(tc.tile_pool(name="vtp", bufs=2))
            lnp = ctx.enter_context(tc.tile_pool(name="lnp", bufs=2))
            ep = ctx.enter_context(tc.tile_pool(name="ep", bufs=4))
            b2p = ctx.enter_context(tc.tile_pool(name="b2p", bufs=2))
            ctxp = ctx.enter_context(tc.tile_pool(name="ctxp", bufs=2))
            stgp = ctx.enter_context(tc.tile_pool(name="stgp", bufs=3))
            ps_st = ctx.enter_context(
                tc.tile_pool(name="ps_st", bufs=2, space="PSUM"))
            ps_ot = ctx.enter_context(
                tc.tile_pool(name="ps_ot", bufs=1, space="PSUM"))
            ps_sh = ctx.enter_context(
                tc.tile_pool(name="ps_sh", bufs=2, space="PSUM"))

            # ---- consts ----
            wq_sb = consts.tile([128, 8, 128], BF16)
            wk_sb = consts.tile([128, 8, 128], BF16)
            wv_sb = consts.tile([128, 8, 128], BF16)
            wo_sb = consts.tile([128, C], BF16)
            msb = consts.tile([128, 4, TQ], BF16)
            gq_sb = consts.tile([128, 1], F32)
            gk_sb = consts.tile([128, 1], F32)
            ones2c = consts.tile([128, 65], BF16)
            bc_sb = consts.tile([128, 128], BF16)
            eps65 = consts.tile([65, 1], F32)
            ident_sb = consts.tile([128, 128], BF16)

            nc.gpsimd.dma_start(out=wq_sb[:], in_=wq_d[:])
            nc.gpsimd.dma_start(out=wk_sb[:], in_=wk_d[:])
            nc.gpsimd.dma_start(out=gq_sb[:], in_=gq_d[:])
            nc.gpsimd.dma_start(out=gk_sb[:], in_=gk_d[:])
            nc.gpsimd.dma_start(out=ones2c[:], in_=ones2c_d[:])
            nc.gpsimd.dma_start(out=bc_sb[:], in_=bc_d[:])
            nc.gpsimd.dma_start(out=wv_sb[:], in_=wv_d[:])
            nc.gpsimd.dma_start(out=msb[:], in_=masks_d[:])
            nc.gpsimd.dma_start(out=wo_sb[:], in_=wo_d[:])
            if not USE_DMA_TRANSPOSE:
                nc.gpsimd.dma_start(out=ident_sb[:], in_=ident_d[:])
            nc.vector.memset(eps65[:], EPS)

            # ---- persistent activations ----
            # chunk 0 arrives as 4 separate ci-pair tiles so each DoubleRow
            # projection matmul depends only on its own pieces' DMAs;
            # chunks 1-3 arrive as one DMA each
            x0p = [acts.tile([128, 2, TQ], BF16, name=f"x0p{i}")
                   for i in range(4)]
            xcs = [None] + [acts.tile([128, 8, TQ], BF16, name=f"xc{c}")
                            for c in range(1, NTQ)]

            def xpiece(c4, ci):
                return (x0p[ci // 2][:, ci % 2, :] if c4 == 0
                        else xcs[c4][:, ci, :])

            QTn = acts.tile([128, T], BF16)
            KZ = [acts.tile([128, T], BF16, name=f"KZ{h}")
                  for h in range(HPC)]
            # per key-tile r: [V0(64) | ones | pad(15) | V1(64) | ones | pad]
            # head blocks at 32B-aligned offsets 0/80 for the DMA xbar
            V_sb = acts.tile([128, NTK, 160], BF16)
            sg128 = acts.tile([128, T], BF16)
            rinv = {"q": acts.tile([128, TQ], BF16, name="rinv_q"),
                    "k": acts.tile([128, TQ], BF16, name="rinv_k")}
            sgf = [acts.tile([1, T], F32, name=f"sgf{h}")
                   for h in range(HPC)]
            sgd = [acts.tile([1, T], F32, name=f"sgd{h}")
                   for h in range(HPC)]

            # zero-init (rows outside the written ranges must stay finite);
            # bulk zeroing arrives via DMA so no compute engine pays for it.
            vview = V_sb[:].rearrange("p r (a b) -> p r a b", b=80)
            nc.vector.memset(vview[:, :, :, 64:65], 1.0)
            nc.vector.memset(rinv["q"][:], 0.0)
            nc.vector.memset(rinv["k"][:], 0.0)

            # ---- input stream ----
            # chunk-0 pieces split across both HWDGE queues (double arrival
            # rate -> denser early matmuls -> HAM warms sooner); zero-fills
            # and chunks 1-3 follow on the scalar queue
            for i in range(4):
                eng = nc.sync if i % 2 == 0 else nc.scalar
                eng.dma_start(out=x0p[i][:], in_=xT_d[:, 0, 2 * i:2 * i + 2])
            nc.scalar.dma_start(out=KZ[0][:], in_=zeros_d[:])
            nc.scalar.dma_start(out=KZ[1][:], in_=zeros_d[:])
            nc.scalar.dma_start(out=xcs[1][:], in_=xT_d[:, 1])
            nc.scalar.dma_start(out=sg128[:], in_=zeros_d[:])
            for c4 in range(2, NTQ):
                nc.scalar.dma_start(out=xcs[c4][:], in_=xT_d[:, c4])

            # ---------- B(c): projections + RMS for chunk c ----------
            # thunk order spreads the serial RMS chains (raw->sq->sums->
            # ln->exp->bb) between independent projection matmuls so the
            # PE queue never head-of-line blocks on a cross-engine dep
            def emit_b(c4):
                sl = slice(TQ * c4, TQ * (c4 + 1))

                def proj_head(w_sb, st):
                    def f():
                        pp = ps_sh.tile([128, TQ], F32, tag="sh", name="pp")
                        st["pp"] = pp
                        for ci in range(4):
                            nc.tensor.matmul(
                                pp[:], w_sb[:, ci, :], xpiece(c4, ci),
                                start=(ci == 0), stop=False)
                    return f

                def proj_tail(w_sb, st):
                    def f():
                        pp = st["pp"]
                        for ci in range(4, 8):
                            nc.tensor.matmul(
                                pp[:], w_sb[:, ci, :], xpiece(c4, ci),
                                start=False, stop=(ci == 7))
                        raw = rawp.tile([128, TQ], BF16, tag="raw",
                                        name="raw")
                        nc.vector.tensor_copy(raw[:], pp[:])
                        sq = sqp.tile([128, TQ], BF16, tag="sq", name="sq")
                        nc.vector.tensor_mul(sq[:], raw[:], raw[:])
                        st["raw"] = raw
                        st["sq"] = sq
                    return f

                def rms(qk, st):
                    def f():
                        sums = ps_sh.tile([65, TQ], F32, tag="sh",
                                          name="sums")
                        nc.tensor.matmul(sums[:], ones2c[:], st["sq"][:],
                                         start=True, stop=True)
                        rln = lnp.tile([65, TQ], F32, tag="ln", name="rln")
                        nc.scalar.activation(rln[:], sums[:], AF.Ln,
                                             bias=eps65[:], scale=1.0 / D)
                        nc.scalar.activation(rinv[qk][0:65, :], rln[:],
                                             AF.Exp, scale=-0.5)
                    return f

                def norm(qk, g_sb, st):
                    def f():
                        bb = ps_sh.tile([128, TQ], F32, tag="sh", name="bb")
                        nc.tensor.matmul(bb[:], bc_sb[:], rinv[qk][:],
                                         start=True, stop=True)
                        raw = st["raw"]
                        if qk == "q":
                            nc.vector.scalar_tensor_tensor(
                                out=QTn[:, sl], in0=raw[:], scalar=g_sb[:],
                                in1=bb[:], op0=MUL, op1=MUL)
                        else:
                            for h in range(HPC):
                                hsl = slice(64 * h, 64 * (h + 1))
                                nc.vector.scalar_tensor_tensor(
                                    out=KZ[h][hsl, sl], in0=raw[hsl, :],
                                    scalar=g_sb[hsl, :], in1=bb[hsl, :],
                                    op0=MUL, op1=MUL)
                    return f

                stq, stk, stv = {}, {}, {}

                def v_head():
                    pv = ps_sh.tile([128, TQ], F32, tag="sh", name="pv")
                    stv["pv"] = pv
                    for ci in range(4):
                        nc.tensor.matmul(
                            pv[:], wv_sb[:, ci, :], xpiece(c4, ci),
                            start=(ci == 0), stop=False)

                def v_tail():
                    pv = stv["pv"]
                    for ci in range(4, 8):
                        nc.tensor.matmul(
                            pv[:], wv_sb[:, ci, :], xpiece(c4, ci),
                            start=False, stop=(ci == 7))
                    vt = vtp.tile([128, TQ], BF16, tag="vt", name="vt")
                    nc.vector.tensor_copy(vt[:], pv[:])
                    stv["vt"] = vt

                def v_tr(rl0):
                    def f():
                        vt = stv["vt"]
                        for rl in (rl0, rl0 + 1):
                            r = 4 * c4 + rl
                            if USE_DMA_TRANSPOSE:
                                for h in range(HPC):
                                    nc.sync.dma_start_transpose(
                                        V_sb[:, r, 80 * h:80 * h + 64],
                                        vt[64 * h:64 * (h + 1),
                                           128 * rl:128 * (rl + 1)])
                            else:
                                tp = ps_sh.tile([128, 128], BF16, tag="sh",
                                                name=f"tp{r}")
                                nc.tensor.transpose(
                                    tp[:], vt[:, 128 * rl:128 * (rl + 1)],
                                    ident_sb[:])
                                dst = V_sb[:, r, :].rearrange(
                                    "p (a b) -> p a b", b=80)[:, :, 0:64]
                                src = tp[:].rearrange("p (a b) -> p a b",
                                                      b=64)
                                nc.vector.tensor_copy(dst, src)
                    return f

                return [proj_head(wq_sb, stq), proj_tail(wq_sb, stq),
                        proj_head(wk_sb, stk), rms("q", stq),
                        proj_tail(wk_sb, stk), norm("q", gq_sb, stq),
                        rms("k", stk), v_head,
                        norm("k", gk_sb, stk), v_tail,
                        v_tr(0), v_tr(2)]

            # ---------- wo(c): normalize + output projection ----------
            def make_prep_h(c4, ot):
                sl = slice(TQ * c4, TQ * (c4 + 1))

                def prep_h(h):
                    # 1/sum_exp for head h, straight off its ot bank; runs
                    # while the other head's groups are still streaming
                    nc.vector.tensor_copy(sgd[h][0:1, sl], ot[h][64:65, :])
                    nc.vector.reciprocal_approx_fast(
                        out=sgf[h][0:1, sl], in_=sgd[h][0:1, sl])
                    nc.vector.tensor_copy(
                        sg128[64 * h:64 * h + 1, sl], sgf[h][0:1, sl])
                return prep_h

            def make_wo_thunks(c4, ot, n_tk):
                sl = slice(TQ * c4, TQ * (c4 + 1))
                st = {}
                thunks = []

                def prep():
                    b2 = ps_sh.tile([128, TQ], F32, tag="sh", name="b2")
                    nc.tensor.matmul(b2[:], bc_sb[:], sg128[:, sl],
                                     start=True, stop=True)
                    b2s = b2p.tile([128, TQ], BF16, tag="b2", name="b2s")
                    nc.vector.tensor_copy(b2s[:], b2[:])
                    ctxT = ctxp.tile([128, TQ], BF16, tag="ctx", name="ctx")
                    for h in range(HPC):
                        hsl = slice(64 * h, 64 * (h + 1))
                        nc.vector.scalar_tensor_tensor(
                            out=ctxT[hsl, :], in0=ot[h][0:64, :], scalar=1.0,
                            in1=b2s[hsl, :], op0=MUL, op1=MUL)
                    st["ctx"] = ctxT

                thunks.append(prep)

                def wo_mu(mu):
                    def f():
                        wop = ps_sh.tile([128, TQ], F32, tag="sh",
                                         name=f"wop{mu}")
                        nc.tensor.matmul(
                            wop[:], wo_sb[:, 128 * mu:128 * (mu + 1)],
                            st["ctx"][:], start=True, stop=True)
                        stg = stgp.tile([128, TQ], BF16, tag="stg",
                                        name=f"stg{mu}")
                        # last chunk's staging is the kernel tail: split it
                        # across ACT (idle after the last exp) and DVE
                        if c4 == NTQ - 1 and mu % 2 == 1:
                            nc.scalar.activation(stg[:], wop[:], AF.Copy)
                        else:
                            nc.vector.tensor_copy(stg[:], wop[:])
                        nc.sync.dma_start(out=outT_d[:, c4, mu, :],
                                          in_=stg[:])
                    return f

                for mu in range(8):
                    thunks.append(wo_mu(mu))
                return thunks

            # ---------- main emission ----------
            for t in emit_b(0):
                t()
            wo_prev = None
            mask_i = 0
            for c4 in range(NTQ):
                sl = slice(TQ * c4, TQ * (c4 + 1))
                n_tk = 4 * (c4 + 1)
                ot = [ps_ot.tile([65, TQ], F32, tag=f"ot{h}",
                                 name=f"ot{h}_{c4}")
                      for h in range(HPC)]

                sched = deque()
                if wo_prev is not None:
                    sched.extend(wo_prev)
                if c4 < NTQ - 1:
                    sched.extend(emit_b(c4 + 1))
                # head-outer: ot[0] completes halfway through the chunk so
                # its denominator prep overlaps head-1's groups
                groups = [(g, h) for h in range(HPC)
                          for g in range(n_tk // 2)]
                per = -(-len(sched) // len(groups))  # ceil division
                prep_h = make_prep_h(c4, ot)

                def emit_pv(g, h, e_t):
                    for j in range(2):
                        r = 2 * g + j
                        nc.tensor.matmul(
                            ot[h][:], V_sb[:, r, 80 * h:80 * h + 65],
                            e_t[:, TQ * j:TQ * (j + 1)],
                            start=(r == 0), stop=(r == n_tk - 1))

                prev = None
                for (g, h) in groups:
                    st_t = ps_st.tile([128, 2 * TQ], F32, tag="st",
                                      name="st")
                    for j in range(2):
                        r = 2 * g + j
                        nc.tensor.matmul(
                            st_t[:, TQ * j:TQ * (j + 1)],
                            KZ[h][:, 128 * r:128 * (r + 1)],
                            QTn[:, sl], start=True, stop=True)
                    e_t = ep.tile([128, 2 * TQ], BF16, tag="e", name="e")
                    nc.scalar.activation(e_t[:], st_t[:], AF.Exp,
                                         scale=SCALE)
                    s0 = 2 * g - 4 * c4
                    if s0 >= 0:
                        ev = e_t[:].rearrange("p (s f) -> p s f", f=TQ)
                        eng = (nc.gpsimd
                               if mask_i % MASK_GP_EVERY == MASK_GP_EVERY - 1
                               else nc.vector)
                        eng.tensor_mul(ev, ev, msb[:, s0:s0 + 2, :])
                        mask_i += 1
                    if prev is not None:
                        emit_pv(*prev)
                        if prev[1] == 0 and h == 1 and g == 0:
                            # queue head-0's denominator prep; it lands a
                            # few groups later so it never preempts the
                            # mask multiplies in the DVE FIFO
                            sched.append(lambda: prep_h(0))
                    prev = (g, h, e_t)
                    for _ in range(per):
                        if sched:
                            sched.popleft()()
                emit_pv(*prev)
                prep_h(1)
                while sched:
                    sched.popleft()()
                wo_prev = make_wo_thunks(c4, ot, n_tk)
            for t in wo_prev:
                t()

    nc.compile()
    return nc


_NC_CACHE = None


def _get_nc():
    global _NC_CACHE
    if _NC_CACHE is None:
        _NC_CACHE = build_nc()
    return _NC_CACHE


def _make_in_maps(x, w_q, w_k, w_v, w_o, q_gamma, k_gamma):
    x = np.asarray(x, dtype=np.float32).reshape(T, C)
    # xT[p, c4, ci, t'] = x[512*c4 + t', 128*ci + p]
    xT = np.ascontiguousarray(
        x.reshape(NTQ, TQ, 8, 128).transpose(3, 0, 2, 1)).astype(NP_BF16)

    p = np.arange(128)
    f = np.arange(TQ)
    masks = np.zeros((128, 4, TQ), dtype=NP_BF16)
    for s in range(4):
        masks[:, s, :] = (f[None, :] >= (p[:, None] + 128 * s)).astype(
            NP_BF16)

    gq = np.tile(np.asarray(q_gamma, np.float32), 2).reshape(128, 1)
    gk = np.tile(np.asarray(k_gamma, np.float32), 2).reshape(128, 1)
    ones2c = np.zeros((128, 65), dtype=NP_BF16)
    ones2c[0:64, 0] = 1
    ones2c[64:128, 64] = 1
    # broadcast stationary: row 0 -> out partitions 0..63,
    # row 64 -> out partitions 64..127
    bc = np.zeros((128, 128), dtype=NP_BF16)
    bc[0, 0:64] = 1
    bc[64, 64:128] = 1
    ident = np.eye(128, dtype=NP_BF16)

    common = dict(xT=xT, masks=masks, gq=gq, gk=gk, ones2c=ones2c, bc=bc,
                  ident=ident, zeros=np.zeros((128, T), dtype=NP_BF16))

    in_maps = []
    for i in range(NCORES):
        rows = slice(JPC * i, JPC * (i + 1))

        def wsw(w):
            # [p, ci, j] = W[rows][j, ci*128+p]
            W = np.asarray(w, np.float32)[rows]           # [128, C]
            return np.ascontiguousarray(
                W.reshape(128, 8, 128).transpose(2, 1, 0)).astype(NP_BF16)

        wo = np.asarray(w_o, np.float32)[:, rows].T        # [128, C]
        in_maps.append(dict(common, wq=wsw(w_q), wk=wsw(w_k), wv=wsw(w_v),
                            wo=np.ascontiguousarray(wo).astype(NP_BF16)))
    return in_maps


def _run(x, w_q, w_k, w_v, w_o, q_gamma, k_gamma, trace=False):
    import time

    from concourse.bass_utils import run_bass_kernel_spmd
    nc = _get_nc()
    in_maps = _make_in_maps(x, w_q, w_k, w_v, w_o, q_gamma, k_gamma)
    res = None
    for attempt in range(3):
        try:
            res = run_bass_kernel_spmd(nc, in_maps, list(range(NCORES)),
                                       trace=trace)
            break
        except Exception:
            # rare transient NRT_EXEC_UNIT_UNRECOVERABLE under axon; the
            # terminal resets the device on the next load
            if attempt == 2:
                raise
            time.sleep(3.0)
    acc = np.zeros((128, NTQ, 8, TQ), dtype=np.float64)
    for r in res.results:
        acc += r["outT"].astype(np.float64)
    # out[512*c4 + t', 128*mu + p] = acc[p, c4, mu, t']
    out = acc.transpose(1, 3, 2, 0).reshape(T, C).astype(np.float32)
    return out.reshape(1, T, C), res


def kernel(x, w_q, w_k, w_v, w_o, q_gamma, k_gamma):
    out, _ = _run(x, w_q, w_k, w_v, w_o, q_gamma, k_gamma, trace=False)
    return out
